# revision 15
# baseline (speedup 1.0000x reference)
"""Distributed causal multi-head attention for 8 TRN2 NeuronCores.

Problem: B=2, T=2048, D=1024, H=16 heads (hd=64), f32 in/out.

Sharding: core i handles batch b=i//4 and head-group g=i%4 (4 heads).
Wq/Wk/Wv column-sharded ([1024, 256] per core), Wo row-sharded
([256, 1024] per core).  Each core computes a partial output projection
for its 4 heads over the full sequence; the host sums the 4 partials
per batch (replacing the all-reduce).  Host pre-casts to bf16 and lays
x out transposed (xT = x^T); partial outputs return as bf16 and are
summed in f32 on the host.

Per-core dataflow (bf16 matmuls, f32 accumulation):
  QT,KT [256(d),2048(t)] = W^T @ x^T  (d on partitions; head pair m at
  partition halves 0:64 / 64:128)
  V     [2048(t),256(d)]              (t on partitions, +ones col)
  scores: for each k-tile, the two heads of a pair issue as a row-tiled
  matmul pair (tile_position rows (0,0)/(64,0), auto-derived from the
  operands' base partitions) running CONCURRENTLY in the PE array --
  the second MM of each pair measures ~4ns on HW -- each into its own
  PSUM bank of a shared [128,1024] tile; one wide exp (ACT) covers
  both heads.  Only the 128-wide diagonal needs an affine_select mask.
  AV: avb[65, q] += V_aug[k,65]^T @ P[k,q]  (col 64 = softmax denom);
  the two heads' accumulation chains interleave per k-tile so each
  consumes exp(kt) as it lands.  Normalization must divide with q on
  partitions (DVE reciprocal is an iterative 8-cycle/element op, so it
  needs a short free dim): avb is PE-transposed per q-tile, recip'd
  [128,4], scaled (per-partition tensor_scalar), and transposed back
  into attnT [dv-pair, t].  out_partial = attnT^T @ Wo (contraction
  128 = 2 heads/plane).

Schedule: ~10 PE warmup matmuls bridge the HAM clock-gate until the
(HBM-bound, all-8-cores-at-once) input DMA lands; inputs arrive as a
few big host-prepacked transfers (4-8KB lines); the exp table preload
sits after the scalar queue's DMA triggers.  Head pairs run in order
(0,0),(1,0),(1,1),(2,0),(2,1),(3,0),(0,1),(3,1) with projections /
V / out-proj spread as PE filler to match each step's exp budget; the
last pair's AV consumes its exps per k-tile, and the final q-tiles'
out-proj casts run on the (by then idle) ACT engine.
"""

import numpy as np
import ml_dtypes

import concourse.bass as bass
import concourse.mybir as mybir
import concourse.tile as tile
from concourse import bacc
from concourse.bass_utils import run_bass_kernel_spmd
from concourse.masks import make_identity

F32 = mybir.dt.float32
BF16 = mybir.dt.bfloat16
AF = mybir.ActivationFunctionType

T = 2048  # sequence length
D = 1024  # embed dim
NH = 4  # heads per core
HD = 64  # head dim
DH = NH * HD  # 256, sharded d per core
TT = T // 128  # 16 t tiles
DT = D // 128  # 8 embed tiles
NSLAB = 4  # q slabs of 512
SCALE = 1.0 / np.sqrt(HD)
N_WARMUP = 10  # PE warmup matmuls: bridge HAM-warm until the first DMA half lands

_NC_CACHE = None


def build():
    nc = bacc.Bacc(None, target_bir_lowering=False, debug=False)

    # host pre-packs inputs into SBUF-ready layouts (4-32KB DMA lines):
    # xT stripe-major [128, 4 stripes x 8 dt x 512], weights [128, dt*DH],
    # wo [128, 2 x D]
    xT_ext = nc.declare_dram_parameter("xT", [128, 4 * DT * 512], BF16, isOutput=False)
    wq = nc.declare_dram_parameter("Wq", [128, DT * DH], BF16, isOutput=False)
    wk = nc.declare_dram_parameter("Wk", [128, DT * DH], BF16, isOutput=False)
    wv = nc.declare_dram_parameter("Wv", [128, DT * DH], BF16, isOutput=False)
    wo = nc.declare_dram_parameter("Wo", [128, 2 * D], BF16, isOutput=False)
    out = nc.declare_dram_parameter("out", [T, D], BF16, isOutput=True)

    with tile.TileContext(nc) as tc:
        with (
            tc.tile_pool(name="persist", bufs=1) as persist,
            tc.tile_pool(name="pt", bufs=2) as pt_pool,
            tc.tile_pool(name="nrm", bufs=2) as nrm_pool,
            tc.tile_pool(name="ev", bufs=2) as ev_pool,
            tc.tile_pool(name="ps_sc", bufs=2, space="PSUM") as ps_sc,
            tc.tile_pool(name="ps_ut", bufs=2, space="PSUM") as ps_ut,
            tc.tile_pool(name="ps_av", bufs=2, space="PSUM") as ps_av,
        ):
            def P(shape, dtype, name):
                return persist.tile(shape, dtype, name=name, tag=name)

            warm = P([128, 512], BF16, "warm")
            wq_bf = P([128, DT * DH], BF16, "wq_bf")
            wk_bf = P([128, DT * DH], BF16, "wk_bf")
            wv_bf = P([128, DT * DH], BF16, "wv_bf")
            wo_bf = P([128, 2 * D], BF16, "wo_bf")
            xT = P([128, DT * T], BF16, "xT")
            QT = P([128, 2 * T], BF16, "QT")
            KT = P([128, 2 * T], BF16, "KT")
            vbuf = P([128, TT * NH * 65], BF16, "vbuf")
            attn = P([128, TT * DH], BF16, "attn")
            attnT = P([128, 2 * T], BF16, "attnT")
            scr = P([1, 8], F32, "scr")
            ident_b = P([128, 128], BF16, "ident_b")
            make_identity(nc, ident_b)
            at3 = attnT.rearrange("p (i t) -> p i t", i=2)

            # ---- t=0: warmup material (gpsimd) ----
            nc.gpsimd.memset(warm[:], 0.25)
            wps = ps_ut.tile([128, 512], F32, name="ut", tag="ut")
            for _ in range(N_WARMUP):
                nc.tensor.matmul(
                    wps[:], lhsT=warm[:, 0:128], rhs=warm[:], start=True, stop=True
                )

            vb3 = vbuf.rearrange("p (t c) -> p t c", c=65)
            nc.gpsimd.memset(vb3[:, :, 64:65], 1.0)
            vb4 = vbuf.rearrange("p (n c) -> p n c", c=65)

            # ---- input DMAs: one multi-dim dma_start each (few triggers
            # -> the scalar/sync sequencers stay free for exp / out-DMAs) ----
            xT3 = xT.rearrange("p (d t) -> p d t", d=DT)
            xE4 = xT_ext.rearrange("p (c d t) -> p c d t", c=4, d=DT)

            def dma_stripe(engn, c, d0=0, d1=DT):
                engn.dma_start(
                    out=xT3[:, d0:d1, c * 512 : (c + 1) * 512],
                    in_=xE4[:, c, d0:d1],
                )

            # halves let the first QK matmuls start per-dt as data lands
            nc.scalar.dma_start(out=wq_bf[:, : 4 * DH], in_=wq[:, : 4 * DH])
            nc.sync.dma_start(out=wq_bf[:, 4 * DH :], in_=wq[:, 4 * DH :])
            dma_stripe(nc.scalar, 0, 0, 4)
            dma_stripe(nc.sync, 0, 4, 8)
            nc.scalar.dma_start(out=wk_bf[:], in_=wk[:])
            nc.sync.dma_start(out=wv_bf[:], in_=wv[:])
            dma_stripe(nc.scalar, 1)
            dma_stripe(nc.sync, 2)
            nc.scalar.dma_start(out=wo_bf[:], in_=wo[:])
            dma_stripe(nc.sync, 3)
            # exp table preload: runs after the scalar queue's DMA triggers,
            # long before the first real exp; input is uninit scratch (the
            # value is irrelevant, only the ACT_TABLE_LOAD matters)
            nc.scalar.activation(
                out=scr[0:1, 0:8], in_=scr[0:1, 0:8], func=AF.Exp, scale=1.0
            )

            # ---- filler thunks ----
            def qk_chunk(w_bf, outT, m, c, cast_eng=None):
                def go():
                    ps = ps_ut.tile([128, 512], F32, name="ut", tag="ut")
                    for dt_ in range(DT):
                        nc.tensor.matmul(
                            ps[:],
                            lhsT=w_bf[
                                :, dt_ * DH + m * 128 : dt_ * DH + (m + 1) * 128
                            ],
                            rhs=xT[:, dt_ * T + c * 512 : dt_ * T + (c + 1) * 512],
                            start=(dt_ == 0),
                            stop=(dt_ == DT - 1),
                        )
                    dst = outT[:, m * T + c * 512 : m * T + (c + 1) * 512]
                    if cast_eng is None:
                        nc.vector.tensor_copy(dst, ps[:])
                    else:
                        cast_eng.copy(dst, ps[:])

                return go

            def v_chunk(tt):
                def go():
                    ps = ps_ut.tile([128, 256], F32, name="ut", tag="ut")
                    for dt_ in range(DT):
                        nc.tensor.matmul(
                            ps[:],
                            lhsT=xT[:, dt_ * T + tt * 128 : dt_ * T + (tt + 1) * 128],
                            rhs=wv_bf[:, dt_ * DH : (dt_ + 1) * DH],
                            start=(dt_ == 0),
                            stop=(dt_ == DT - 1),
                        )
                    nc.vector.tensor_copy(
                        vb4[:, tt * NH : (tt + 1) * NH, 0:64],
                        ps.rearrange("p (n c) -> p n c", n=NH),
                    )

                return go

            def op_chunk(tt, cast_eng=None, split_dma=False):
                def go():
                    ev = ev_pool.tile([128, 1024], BF16, name="ev", tag="ev")
                    for ec in range(2):
                        ps = ps_ut.tile([128, 512], F32, name="ut", tag="ut")
                        for i in range(2):
                            nc.tensor.matmul(
                                ps[:],
                                lhsT=attnT[
                                    :, i * T + tt * 128 : i * T + (tt + 1) * 128
                                ],
                                rhs=wo_bf[:, i * D + ec * 512 : i * D + (ec + 1) * 512],
                                start=(i == 0),
                                stop=(i == 1),
                            )
                        if cast_eng is None:
                            nc.vector.tensor_copy(
                                ev[:, ec * 512 : (ec + 1) * 512], ps[:]
                            )
                        else:
                            cast_eng.copy(
                                ev[:, ec * 512 : (ec + 1) * 512], ps[:]
                            )
                        if split_dma:
                            nc.sync.dma_start(
                                out=out[
                                    tt * 128 : (tt + 1) * 128,
                                    ec * 512 : (ec + 1) * 512,
                                ],
                                in_=ev[:, ec * 512 : (ec + 1) * 512],
                            )
                    if not split_dma:
                        nc.sync.dma_start(
                            out=out[tt * 128 : (tt + 1) * 128, :], in_=ev[:]
                        )

                return go

            # ---- scores (row-tiled head pair) + exp, per k-tile ----
            def scores_chunks(s, m, pt):
                thunks = []
                for kt in range(4 * (s + 1)):
                    j = kt - 4 * s
                    o = 128 * j if j > 0 else 0
                    w = 512 - o

                    def go(kt=kt, j=j, o=o, w=w):
                        ps = ps_sc.tile([128, 1024], F32, name="sc", tag="sc")
                        for r in range(2):
                            r0 = r * 64
                            nc.tensor.matmul(
                                ps[:, r * 512 : r * 512 + w],
                                lhsT=KT[
                                    r0 : r0 + 64,
                                    m * T + kt * 128 : m * T + (kt + 1) * 128,
                                ],
                                rhs=QT[
                                    r0 : r0 + 64,
                                    m * T + s * 512 + o : m * T + (s + 1) * 512,
                                ],
                                start=True,
                                stop=True,
                            )
                        nc.scalar.activation(
                            out=pt[:, kt * 1024 : kt * 1024 + 512 + w],
                            in_=ps[:, 0 : 512 + w],
                            func=AF.Exp,
                            scale=float(SCALE),
                        )
                        if j >= 0:
                            for r in range(2):
                                nc.gpsimd.affine_select(
                                    out=pt[
                                        :, kt * 1024 + r * 512 : kt * 1024 + r * 512 + 128
                                    ],
                                    in_=pt[
                                        :, kt * 1024 + r * 512 : kt * 1024 + r * 512 + 128
                                    ],
                                    pattern=[[1, 128]],
                                    compare_op=mybir.AluOpType.is_ge,
                                    fill=0.0,
                                    base=0,
                                    channel_multiplier=-1,
                                )

                    thunks.append(go)
                return thunks

            # ---- AV + normalize (transpose epilogue) for one pair ----
            # Division must happen with q on partitions (DVE reciprocal is
            # an 8-cycle/element iterative op -> needs a short free dim), so
            # avb [d, q] is PE-transposed to [q, d], normalized with a
            # per-partition scalar multiply, and transposed back per q-tile.
            def av_ops(s, m, pt):
                nk = 4 * (s + 1)

                def offw(kt):
                    j = kt - 4 * s
                    o = 128 * j if j > 0 else 0
                    return o, 512 - o

                state = {}

                def av_chains():
                    """Both heads' AV chains interleaved per k-tile, so each
                    consumes exp(kt) as it lands and both finish together."""

                    def go():
                        avbs = []
                        for h01 in range(2):
                            avb = ps_av.tile(
                                [128, 512], F32, name="avb", tag="av"
                            )
                            state[f"avb{h01}"] = avb
                            avbs.append(avb)
                        for kt in range(nk):
                            o, w = offw(kt)
                            for h01 in range(2):
                                nc.tensor.matmul(
                                    avbs[h01][0:65, o:512],
                                    lhsT=vb4[:, kt * NH + 2 * m + h01, :],
                                    rhs=pt[
                                        :,
                                        kt * 1024 + 512 * h01 : kt * 1024
                                        + 512 * h01
                                        + w,
                                    ],
                                    start=(kt == 0),
                                    stop=(kt == nk - 1),
                                )
                        for h01 in range(2):
                            st = nrm_pool.tile(
                                [65, 512], BF16, name="st", tag="st"
                            )
                            state[f"st{h01}"] = st
                            nc.vector.tensor_copy(
                                st[:], avbs[h01][0:65, :]
                            )

                    return go

                def tr(h01):
                    def go():
                        st = state[f"st{h01}"]
                        pn = ps_av.tile([128, 264], BF16, name="pn", tag="av")
                        for qi in range(4):
                            nc.tensor.transpose(
                                pn[:, qi * 66 : qi * 66 + 65],
                                st[:, qi * 128 : (qi + 1) * 128],
                                ident_b[0:65, 0:65],
                            )
                        rc = nrm_pool.tile([128, 4], F32, name="rc", tag="rc")
                        nc.vector.reciprocal(
                            rc[:],
                            pn.rearrange("p (n c) -> p n c", c=66)[:, :, 64],
                        )
                        state[f"pn{h01}"], state[f"rc{h01}"] = pn, rc

                    return go

                def norm(h01):
                    def go():
                        pn, rc = state[f"pn{h01}"], state[f"rc{h01}"]
                        h = 2 * m + h01
                        for qi in range(4):
                            qt = 4 * s + qi
                            nc.vector.tensor_scalar_mul(
                                attn[:, qt * DH + h * 64 : qt * DH + (h + 1) * 64],
                                pn[:, qi * 66 : qi * 66 + 64],
                                rc[:, qi : qi + 1],
                            )

                    return go

                return [
                    av_chains(),
                    tr(0),
                    tr(1),
                    norm(0),
                    norm(1),
                ]

            # attnT transposes for one q-tile (after both planes normalized)
            def tr_chunk(qt):
                def go():
                    ps = ps_av.tile([128, 256], BF16, name="trb", tag="av")
                    for i in range(2):
                        nc.tensor.transpose(
                            ps[:, i * 128 : (i + 1) * 128],
                            attn[:, qt * DH + i * 128 : qt * DH + (i + 1) * 128],
                            ident_b[:],
                        )
                    nc.vector.tensor_copy(
                        at3[:, :, qt * 128 : (qt + 1) * 128],
                        ps.rearrange("p (i c) -> p i c", i=2),
                    )

                return go

            def interleave(a, b):
                if not a:
                    return list(b)
                if not b:
                    return list(a)
                res = []
                nb, na, bi = len(b), len(a), 0
                for i, op in enumerate(a):
                    res.append(op)
                    want = (i + 1) * nb // na
                    while bi < want:
                        res.append(b[bi])
                        bi += 1
                res.extend(b[bi:])
                return res

            # ---- static filler plan per pair step ----
            QC = {
                (mt, m, c): qk_chunk(w, o, m, c)
                for mt, w, o in (("Q", wq_bf, QT), ("K", wk_bf, KT))
                for m in range(2)
                for c in range(4)
            }
            fillers = {
                0: [QC[("Q", 0, 1)], QC[("K", 0, 1)]]
                + [v_chunk(t) for t in range(4)],
                1: [QC[("Q", 1, 0)], QC[("K", 1, 0)], QC[("Q", 1, 1)], QC[("K", 1, 1)]]
                + [v_chunk(t) for t in range(4, 8)],
                2: [QC[("Q", 0, 2)], QC[("K", 0, 2)]],
                3: [QC[("Q", 1, 2)], QC[("K", 1, 2)]]
                + [v_chunk(t) for t in range(8, 12)],
                4: [QC[("Q", 0, 3)], QC[("K", 0, 3)]]
                + [tr_chunk(t) for t in range(4, 8)]
                + [op_chunk(4), op_chunk(5)],
                5: [QC[("Q", 1, 3)], QC[("K", 1, 3)]]
                + [v_chunk(t) for t in range(12, 16)]
                + [tr_chunk(t) for t in range(8, 12)]
                + [op_chunk(6), op_chunk(7)],
                6: [op_chunk(8), op_chunk(9), op_chunk(10), op_chunk(11)],
                7: [tr_chunk(t) for t in range(0, 4)]
                + [op_chunk(t) for t in range(0, 4)],
            }

            # ---- prologue: first QK chunks for pair (0,0); their casts
            # run on the (idle until first exp) scalar engine so the first
            # scores do not wait on the DVE queue spin-up ----
            qk_chunk(wq_bf, QT, 0, 0, cast_eng=nc.scalar)()
            qk_chunk(wk_bf, KT, 0, 0, cast_eng=nc.scalar)()

            pairs = [(0, 0), (1, 0), (1, 1), (2, 0), (2, 1), (3, 0), (0, 1), (3, 1)]
            pts = {}
            prev = None
            for idx in range(len(pairs) + 1):
                sc = []
                if idx < len(pairs):
                    s, m = pairs[idx]
                    pts[idx] = pt_pool.tile(
                        [128, 4 * (s + 1) * 1024], BF16, name="pt", tag="pt"
                    )
                    sc = scores_chunks(s, m, pts[idx])
                av = []
                if prev is not None:
                    ps_, pm_ = pairs[prev]
                    av = av_ops(ps_, pm_, pts[prev])
                if idx == len(pairs):
                    # tail: AV(3,1) consumes exps per-k-tile as they land,
                    # then slab-3 transpose + out-proj per qt
                    av[0]()  # interleaved AV chains + st copies
                    av[1]()  # pn transposes + recip (even)
                    av[2]()  # pn transposes + recip (odd)
                    av[3]()  # norm even (DVE)
                    av[4]()  # norm odd (DVE)
                    for t in range(12, 16):
                        tr_chunk(t)()
                        op_chunk(t, cast_eng=nc.scalar, split_dma=True)()
                else:
                    for opf in interleave(sc, av + fillers.get(idx, [])):
                        opf()
                prev = idx

    nc.compile()
    return nc


def _get_nc():
    global _NC_CACHE
    if _NC_CACHE is None:
        _NC_CACHE = build()
    return _NC_CACHE


def _pack_w(w):
    # [1024, 256] -> [128, 8*256]: row p = concat over dt of w[dt*128+p, :]
    return np.ascontiguousarray(
        w.reshape(DT, 128, DH).transpose(1, 0, 2).reshape(128, DT * DH)
    )


def _pack_wo(w):
    # [256, 1024] -> [128, 2*1024]
    return np.ascontiguousarray(
        w.reshape(2, 128, D).transpose(1, 0, 2).reshape(128, 2 * D)
    )


def _pack_x(xs):
    # x^T [1024, 2048] -> stripe-major [128, 4*8*512]
    return np.ascontiguousarray(
        xs.reshape(DT, 128, 4, 512).transpose(1, 2, 0, 3).reshape(128, 4 * DT * 512)
    )


def make_in_maps(x, Wq, Wk, Wv, Wo):
    bf = ml_dtypes.bfloat16
    x = np.asarray(x, dtype=np.float32)
    WqT = np.asarray(Wq, dtype=np.float32).astype(bf)
    WkT = np.asarray(Wk, dtype=np.float32).astype(bf)
    WvT = np.asarray(Wv, dtype=np.float32).astype(bf)
    WoT = np.asarray(Wo, dtype=np.float32).astype(bf)
    xTb = [_pack_x(x[b].T.astype(bf)) for b in range(2)]
    in_maps = []
    for core in range(8):
        b, g = core // 4, core % 4
        sl = slice(g * DH, (g + 1) * DH)
        in_maps.append(
            {
                "xT": xTb[b],
                "Wq": _pack_w(WqT[:, sl]),
                "Wk": _pack_w(WkT[:, sl]),
                "Wv": _pack_w(WvT[:, sl]),
                "Wo": _pack_wo(WoT[sl, :]),
            }
        )
    return in_maps


def unshard(results):
    out = np.empty((2, T, D), np.float32)
    for b in range(2):
        acc = results[4 * b]["out"].astype(np.float32)
        for g in range(1, 4):
            acc += results[4 * b + g]["out"].astype(np.float32)
        out[b] = acc
    return out


def kernel(x, Wq, Wk, Wv, Wo):
    nc = _get_nc()
    in_maps = make_in_maps(x, Wq, Wk, Wv, Wo)
    res = run_bass_kernel_spmd(nc, in_maps, core_ids=list(range(8)))
    return unshard(res.results)
